# revision 18
# baseline (speedup 1.0000x reference)
"""Trainium2 Bass kernel for nn_AttentionBlock (B=4, C=H=W=S=256). v4

reference:
  q = Wq @ query + bq   (1x1 conv over channel dim)
  k = Wk @ key_in + bk
  v = Wv @ value + bv
  scores[b,i,h,w] = sum_j q[b,i,h,j] * k[b,j,i,w]
  attn = softmax(scores, -1)
  out[b,i,h,w] = sum_j attn[b,i,h,j] * v[b,i,j,w]
  return sigmoid(out)

Sharding: 8 cores = (b, g) with b=core//2, g=core%2; each core computes
out[b, g*128:(g+1)*128, :, :].

vs baseline (DMA-bound at ~260MB HBM traffic per core):
  - all streamed I/O is 16-bit: inputs host-cast to f16, output written
    f16 and upcast on host (inputs 160->80MB, output 33.5->16.8MB)
  - q never round-trips DRAM: the conv result is transposed in 128x128
    blocks on the tensor engine (identity matmul) during phase A and
    kept SBUF-resident as qT[j, jb, i, h] f16 (16.8MB)
  - only v round-trips DRAM (bf16, natural-layout readback)
  - sigmoid(y) is computed as 0.5*tanh(y/2)+0.5: tanh shares the ACT
    function table with exp, so the scalar engine never reloads its
    1283ns act table inside the attention loop (sigmoid does not share
    a table with exp). The /2 is folded into the softmax-denominator
    column of v (2.0 instead of 1.0 -> reciprocal gives 0.5/rowsum),
    and the *0.5+0.5 affine runs on the otherwise-idle Pool engine.
  - PSUM drains alternate between the two PSUM-capable elementwise
    engines (q->DVE / v->ACT per subchunk, k and the qT copybacks
    split likewise) so the conv PSUM ring turns over at 2x the
    single-engine drain rate and the PE stays continuously busy
  - softmax normalization is a DVE pre-scale (yt = po * 0.5/rowsum)
    followed by ONE batched tanh per chunk and an in-place Pool affine
  - all large DMAs are issued on the two HWDGE queues (SP + ACT) in
    few, large instructions (per-DMA fixed cost ~1.3us); v readback
    covers 4 i's per instruction; key-input chunks prefetched 2 ahead
  => ~130MB HBM traffic; PE/ACT/DVE modeled busy ~520/460/430us.
"""

import numpy as np

import concourse.bass as bass
import concourse.tile as tile
from concourse import bacc, mybir
from concourse.bass_utils import run_bass_kernel_spmd
from concourse.masks import make_identity

C = 256
HALF = 128          # output channels per core
N_CORES = 8
ICHUNK = 4          # i values per phase-B chunk
A_CHUNK = 1024      # flattened spatial elems per phase-A chunk

_CACHE = {}


def build_nc(repeat=1):
    key = ("nc", repeat)
    if key in _CACHE:
        return _CACHE[key]
    f32 = mybir.dt.float32
    f16 = mybir.dt.float16
    bf16 = mybir.dt.bfloat16
    Ident = mybir.ActivationFunctionType.Identity

    nc = bacc.Bacc("TRN2", target_bir_lowering=False, debug=False,
                   num_devices=N_CORES)

    query_b = nc.dram_tensor("query_b", [C, C, C], f16, kind="ExternalInput").ap()
    key_h = nc.dram_tensor("key_h", [C, HALF, C], f16, kind="ExternalInput").ap()
    value_b = nc.dram_tensor("value_b", [C, C, C], f16, kind="ExternalInput").ap()
    wqT = nc.dram_tensor("wqT", [C, HALF], f16, kind="ExternalInput").ap()
    wkT = nc.dram_tensor("wkT", [C, C], f16, kind="ExternalInput").ap()
    wvT = nc.dram_tensor("wvT", [C, HALF], f16, kind="ExternalInput").ap()
    bq_h = nc.dram_tensor("bq_h", [HALF, 1], f32, kind="ExternalInput").ap()
    bk_f = nc.dram_tensor("bk_f", [C, 1], f32, kind="ExternalInput").ap()
    bv_h = nc.dram_tensor("bv_h", [HALF, 1], f32, kind="ExternalInput").ap()
    out_b = nc.dram_tensor("out_b", [HALF, C, C], f16, kind="ExternalOutput").ap()

    v_scr = nc.dram_tensor("v_scr", [HALF, C, C], bf16).ap()

    HJ = C * C  # 65536
    KCH = ICHUNK * C  # flattened (il, w) per phase-B chunk

    qv_in = query_b.rearrange("(cb c) h j -> c cb (h j)", c=128)
    vv_in = value_b.rearrange("(cb c) j w -> c cb (j w)", c=128)
    kv_in = key_h.rearrange("(cb c) il w -> c cb (il w)", c=128)
    v_flat = v_scr.rearrange("i j w -> i (j w)")

    NSUB = A_CHUNK // 512  # 512-wide matmul subchunks per phase-A chunk

    with tile.TileContext(nc) as tc:
        with (
            tc.tile_pool(name="weights", bufs=1) as wpool,
            tc.tile_pool(name="a_in", bufs=2) as a_in,
            tc.tile_pool(name="a_qs", bufs=2) as a_qs,
            tc.tile_pool(name="a_vs", bufs=2) as a_vs,
            tc.tile_pool(name="ps512", bufs=2, space="PSUM") as ps512,
            tc.tile_pool(name="psT", bufs=2, space="PSUM") as psT_pool,
            tc.tile_pool(name="b_kin", bufs=2) as b_kin,
            tc.tile_pool(name="b_ksb", bufs=2) as b_ksb,
            tc.tile_pool(name="b_vt", bufs=2) as b_vt,
            tc.tile_pool(name="b_et", bufs=2) as b_et,
            tc.tile_pool(name="b_ob", bufs=2) as b_ob,
            tc.tile_pool(name="b_rs", bufs=4) as b_rs,
            tc.tile_pool(name="b_psc", bufs=2, space="PSUM") as b_psc,
            tc.tile_pool(name="b_po", bufs=2, space="PSUM") as b_po,
        ):
            wq_sb = wpool.tile([128, 2, HALF], f16)
            wk_sb = wpool.tile([128, 2, C], f16)
            wv_sb = wpool.tile([128, 2, HALF], f16)
            nc.gpsimd.dma_start(out=wq_sb, in_=wqT.rearrange("(cb c) i -> c cb i", c=128))
            nc.gpsimd.dma_start(out=wk_sb, in_=wkT.rearrange("(cb c) j -> c cb j", c=128))
            nc.gpsimd.dma_start(out=wv_sb, in_=wvT.rearrange("(cb c) i -> c cb i", c=128))
            sb_bq = wpool.tile([128, 1], f32)
            sb_bk = wpool.tile([128, 2, 1], f32)
            sb_bv = wpool.tile([128, 1], f32)
            nc.gpsimd.dma_start(out=sb_bq, in_=bq_h)
            nc.gpsimd.dma_start(out=sb_bk, in_=bk_f.rearrange("(jb j) one -> j jb one", j=128))
            nc.gpsimd.dma_start(out=sb_bv, in_=bv_h)
            ident = wpool.tile([128, 128], f16)
            make_identity(nc, ident)
            # SBUF-resident transposed q for the whole kernel: qT[j, jb, i, h]
            qT = wpool.tile([128, 2, HALF, C], f16)

            def body(_it=None):
                # ---------------- Phase A: q conv -> resident qT; v conv ------
                for t in range(HJ // A_CHUNK):
                    sl = slice(t * A_CHUNK, (t + 1) * A_CHUNK)
                    qc = a_in.tile([128, 2, A_CHUNK], f16, tag="qc")
                    nc.sync.dma_start(out=qc, in_=qv_in[:, :, sl])
                    vc = a_in.tile([128, 2, A_CHUNK], f16, tag="vc")
                    nc.sync.dma_start(out=vc, in_=vv_in[:, :, sl])
                    # q and v subchunks alternate so their PSUM drains
                    # land on different engines (q->DVE, v->ACT): the ps512
                    # ring turns over at 2x the single-engine drain rate and
                    # the PE stays continuously busy (p-state)
                    qss = []
                    vs = a_vs.tile([128, A_CHUNK], bf16, tag="vs")
                    for n in range(NSUB):
                        ps = ps512.tile([128, 512], f32, tag="aps")
                        for cb in range(2):
                            nc.tensor.matmul(ps, wq_sb[:, cb, :],
                                             qc[:, cb, n * 512:(n + 1) * 512],
                                             start=(cb == 0), stop=(cb == 1))
                        qs = a_qs.tile([128, 512], f16, tag="qs")
                        nc.vector.tensor_scalar(
                            out=qs, in0=ps, scalar1=sb_bq, scalar2=None,
                            op0=mybir.AluOpType.add)
                        qss.append(qs)
                        ps = ps512.tile([128, 512], f32, tag="aps")
                        for cb in range(2):
                            nc.tensor.matmul(ps, wv_sb[:, cb, :],
                                             vc[:, cb, n * 512:(n + 1) * 512],
                                             start=(cb == 0), stop=(cb == 1))
                        nc.scalar.activation(
                            out=vs[:, n * 512:(n + 1) * 512], in_=ps,
                            func=Ident, bias=sb_bv)
                    nc.sync.dma_start(out=v_flat[:, sl], in_=vs)
                    for n in range(NSUB):
                        pt = psT_pool.tile([128, 2, 2, 128], f16, tag="pt")
                        for h2 in range(2):
                            for jb in range(2):
                                nc.tensor.transpose(
                                    pt[:, h2, jb, :],
                                    qss[n][:, h2 * 256 + jb * 128:
                                           h2 * 256 + (jb + 1) * 128],
                                    ident)
                        h0 = (t * NSUB + n) * 2
                        # copyback split across the two PSUM-capable engines;
                        # each copy covers both jb blocks: free dims (jb, i)
                        nc.vector.tensor_copy(
                            out=qT[:, :, :, h0],
                            in_=pt[:, 0, :, :])
                        nc.scalar.copy(
                            out=qT[:, :, :, h0 + 1],
                            in_=pt[:, 1, :, :])

                # ---------------- Phase B: k conv + attention -----------------
                def kc_load(ic):
                    kc = b_kin.tile([128, 2, KCH], f16, tag="kc")
                    nc.gpsimd.dma_start(
                        out=kc, in_=kv_in[:, :, ic * KCH:(ic + 1) * KCH])
                    return kc
                def vt_load(ic):
                    i0 = ic * ICHUNK
                    vt = b_vt.tile([128, ICHUNK, 2, C + 8], bf16, tag="vt")
                    nc.sync.dma_start(
                        out=vt[:, :, :, 0:C],
                        in_=v_scr[i0:i0 + ICHUNK].rearrange(
                            "io (jb j) w -> j io jb w", j=128))
                    nc.gpsimd.memset(vt[:, :, :, C:C + 1], 2.0)
                    return vt

                kc_pend = {0: kc_load(0), 1: kc_load(1)}
                vt_pend = {0: vt_load(0)}
                for ic in range(HALF // ICHUNK):
                    i0 = ic * ICHUNK
                    kc = kc_pend.pop(ic)
                    if ic + 2 < HALF // ICHUNK:
                        kc_pend[ic + 2] = kc_load(ic + 2)
                    vt = vt_pend.pop(ic)
                    if ic + 1 < HALF // ICHUNK:
                        vt_pend[ic + 1] = vt_load(ic + 1)
                    ksb = b_ksb.tile([128, 2, ICHUNK, C], f16, tag="ksb")
                    for jb in range(2):
                        for n in range(KCH // 512):
                            ps = ps512.tile([128, 512], f32, tag="aps")
                            for cb in range(2):
                                nc.tensor.matmul(
                                    ps, wk_sb[:, cb, jb * 128:(jb + 1) * 128],
                                    kc[:, cb, n * 512:(n + 1) * 512],
                                    start=(cb == 0), stop=(cb == 1))
                            # k drains alternate DVE/ACT (Identity is in
                            # every act table set - no table reload)
                            if jb == 0:
                                nc.vector.tensor_scalar(
                                    out=ksb[:, jb, n * 2:(n + 1) * 2, :],
                                    in0=ps, scalar1=sb_bk[:, jb, :],
                                    scalar2=None, op0=mybir.AluOpType.add)
                            else:
                                nc.scalar.activation(
                                    out=ksb[:, jb, n * 2:(n + 1) * 2, :],
                                    in_=ps, func=Ident, bias=sb_bk[:, jb, :])
                    ob = b_ob.tile([128, ICHUNK, 2, C], f16, tag="ob")
                    stage = []
                    for io in range(ICHUNK):
                        i_loc = i0 + io
                        psc = b_psc.tile([128, 2, C], f32, tag="psc")
                        for wb in range(2):
                            for jb in range(2):
                                nc.tensor.matmul(
                                    psc[:, wb, :],
                                    ksb[:, jb, io, wb * 128:(wb + 1) * 128],
                                    qT[:, jb, i_loc, :],
                                    start=(jb == 0), stop=(jb == 1))
                        et = b_et.tile([128, 2, C], bf16, tag="et")
                        nc.scalar.activation(
                            out=et, in_=psc,
                            func=mybir.ActivationFunctionType.Exp)
                        stage.append((io, et))
                    for io, et in stage:
                        for hb in range(2):
                            po = b_po.tile([128, C + 1], f32, tag="po")
                            for wb in range(2):
                                nc.tensor.matmul(
                                    po, et[:, wb, hb * 128:(hb + 1) * 128],
                                    vt[:, io, wb, 0:C + 1],
                                    start=(wb == 0), stop=(wb == 1))
                            rs = b_rs.tile([128, 1], f32, tag="rs")
                            nc.vector.reciprocal(out=rs, in_=po[:, C:C + 1])
                            # sigmoid(y) = 0.5*tanh(y/2)+0.5, y/2 = po*rs
                            # (rs = 0.5/rowsum via vt's 2.0-column; bv is
                            # already in v from the phase-A drain). One
                            # scaled tanh per (io,hb) replaces the DVE
                            # normalize pass + batched tanh.
                            nc.scalar.activation(
                                out=ob[:, io, hb, :], in_=po[:, 0:C],
                                func=mybir.ActivationFunctionType.Tanh,
                                scale=rs)
                    nc.gpsimd.tensor_scalar(
                        out=ob, in0=ob,
                        scalar1=0.5, scalar2=0.5,
                        op0=mybir.AluOpType.mult,
                        op1=mybir.AluOpType.add)
                    nc.gpsimd.dma_start(
                        out=out_b[i0:i0 + ICHUNK].rearrange(
                            "io (hb h) w -> h io hb w", h=128),
                        in_=ob)

            if repeat == 1:
                body()
            else:
                with tc.For_i(0, repeat, 1) as it:
                    body(it)

    nc.compile()
    _CACHE[key] = nc
    return nc


def make_in_maps(inputs):
    query = np.asarray(inputs["query"], dtype=np.float32)
    key_in = np.asarray(inputs["key_in"], dtype=np.float32)
    value = np.asarray(inputs["value"], dtype=np.float32)
    Wq = np.asarray(inputs["Wq"], dtype=np.float32)
    Wk = np.asarray(inputs["Wk"], dtype=np.float32)
    Wv = np.asarray(inputs["Wv"], dtype=np.float32)
    bq = np.asarray(inputs["bq"], dtype=np.float32)
    bk = np.asarray(inputs["bk"], dtype=np.float32)
    bv = np.asarray(inputs["bv"], dtype=np.float32)
    in_maps = []
    for core in range(N_CORES):
        b, g = core // 2, core % 2
        sl = slice(g * HALF, (g + 1) * HALF)
        in_maps.append({
            "query_b": np.ascontiguousarray(query[b], dtype=np.float16),
            "key_h": np.ascontiguousarray(key_in[b][:, sl, :], dtype=np.float16),
            "value_b": np.ascontiguousarray(value[b], dtype=np.float16),
            "wqT": np.ascontiguousarray(Wq[sl, :].T, dtype=np.float16),
            "wkT": np.ascontiguousarray(Wk.T, dtype=np.float16),
            "wvT": np.ascontiguousarray(Wv[sl, :].T, dtype=np.float16),
            "bq_h": np.ascontiguousarray(bq[sl].reshape(HALF, 1)),
            "bk_f": np.ascontiguousarray(bk.reshape(C, 1)),
            "bv_h": np.ascontiguousarray(bv[sl].reshape(HALF, 1)),
        })
    return in_maps


def kernel(query, key_in, value, Wq, bq, Wk, bk, Wv, bv):
    nc = build_nc()
    in_maps = make_in_maps(dict(query=query, key_in=key_in, value=value,
                                Wq=Wq, bq=bq, Wk=Wk, bk=bk, Wv=Wv, bv=bv))
    res = run_bass_kernel_spmd(nc, in_maps, core_ids=list(range(N_CORES)))
    out = np.empty((4, C, C, C), dtype=np.float32)
    for core in range(N_CORES):
        b, g = core // 2, core % 2
        out[b, g * HALF:(g + 1) * HALF] = res.results[core]["out_b"].astype(np.float32)
    return out


# revision 19
# speedup vs baseline: 1.4065x; 1.4065x over previous
"""Trainium2 Bass kernel for nn_AttentionBlock (B=4, C=H=W=S=256). v4

reference:
  q = Wq @ query + bq   (1x1 conv over channel dim)
  k = Wk @ key_in + bk
  v = Wv @ value + bv
  scores[b,i,h,w] = sum_j q[b,i,h,j] * k[b,j,i,w]
  attn = softmax(scores, -1)
  out[b,i,h,w] = sum_j attn[b,i,h,j] * v[b,i,j,w]
  return sigmoid(out)

Sharding: 8 cores = (b, g) with b=core//2, g=core%2; each core computes
out[b, g*128:(g+1)*128, :, :].

vs baseline (DMA-bound at ~260MB HBM traffic per core):
  - all streamed I/O is 16-bit: inputs host-cast to f16, output written
    f16 and upcast on host (inputs 160->80MB, output 33.5->16.8MB)
  - q never round-trips DRAM: the conv result is transposed in 128x128
    blocks on the tensor engine (identity matmul) during phase A and
    kept SBUF-resident as qT[j, jb, i, h] f16 (16.8MB)
  - only v round-trips DRAM (bf16, natural-layout readback)
  - sigmoid(y) is computed as 0.5*tanh(y/2)+0.5: tanh shares the ACT
    function table with exp, so the scalar engine never reloads its
    1283ns act table inside the attention loop (sigmoid does not share
    a table with exp). The /2 is folded into the softmax-denominator
    column of v (2.0 instead of 1.0 -> reciprocal gives 0.5/rowsum),
    and the *0.5+0.5 affine runs on the otherwise-idle Pool engine.
  - PSUM drains alternate between the two PSUM-capable elementwise
    engines (q->DVE / v->ACT per subchunk, k and the qT copybacks
    split likewise) so the conv PSUM ring turns over at 2x the
    single-engine drain rate and the PE stays continuously busy
  - softmax normalization is a DVE pre-scale (yt = po * 0.5/rowsum)
    followed by ONE batched tanh per chunk and an in-place Pool affine
  - all large DMAs are issued on the two HWDGE queues (SP + ACT) in
    few, large instructions (per-DMA fixed cost ~1.3us); v readback
    covers 4 i's per instruction; key-input chunks prefetched 2 ahead
  => ~130MB HBM traffic; PE/ACT/DVE modeled busy ~520/460/430us.
"""

import numpy as np

import concourse.bass as bass
import concourse.tile as tile
from concourse import bacc, mybir
from concourse.bass_utils import run_bass_kernel_spmd
from concourse.masks import make_identity

C = 256
HALF = 128          # output channels per core
N_CORES = 8
ICHUNK = 4          # i values per phase-B chunk
A_CHUNK = 1024      # flattened spatial elems per phase-A chunk

_CACHE = {}


def build_nc(repeat=1):
    key = ("nc", repeat)
    if key in _CACHE:
        return _CACHE[key]
    f32 = mybir.dt.float32
    f16 = mybir.dt.float16
    bf16 = mybir.dt.bfloat16
    Ident = mybir.ActivationFunctionType.Identity

    nc = bacc.Bacc("TRN2", target_bir_lowering=False, debug=False,
                   num_devices=N_CORES)

    query_b = nc.dram_tensor("query_b", [C, C, C], f16, kind="ExternalInput").ap()
    key_h = nc.dram_tensor("key_h", [C, HALF, C], f16, kind="ExternalInput").ap()
    value_b = nc.dram_tensor("value_b", [C, C, C], f16, kind="ExternalInput").ap()
    wqT = nc.dram_tensor("wqT", [C, HALF], f16, kind="ExternalInput").ap()
    wkT = nc.dram_tensor("wkT", [C, C], f16, kind="ExternalInput").ap()
    wvT = nc.dram_tensor("wvT", [C, HALF], f16, kind="ExternalInput").ap()
    bq_h = nc.dram_tensor("bq_h", [HALF, 1], f32, kind="ExternalInput").ap()
    bk_f = nc.dram_tensor("bk_f", [C, 1], f32, kind="ExternalInput").ap()
    bv_h = nc.dram_tensor("bv_h", [HALF, 1], f32, kind="ExternalInput").ap()
    out_b = nc.dram_tensor("out_b", [HALF, C, C], f16, kind="ExternalOutput").ap()

    v_scr = nc.dram_tensor("v_scr", [HALF, C, C], bf16).ap()

    HJ = C * C  # 65536
    KCH = ICHUNK * C  # flattened (il, w) per phase-B chunk

    qv_in = query_b.rearrange("(cb c) h j -> c cb (h j)", c=128)
    vv_in = value_b.rearrange("(cb c) j w -> c cb (j w)", c=128)
    kv_in = key_h.rearrange("(cb c) il w -> c cb (il w)", c=128)
    v_flat = v_scr.rearrange("i j w -> i (j w)")

    NSUB = A_CHUNK // 512  # 512-wide matmul subchunks per phase-A chunk

    with tile.TileContext(nc) as tc:
        with (
            tc.tile_pool(name="weights", bufs=1) as wpool,
            tc.tile_pool(name="a_in", bufs=2) as a_in,
            tc.tile_pool(name="a_qs", bufs=2) as a_qs,
            tc.tile_pool(name="a_vs", bufs=2) as a_vs,
            tc.tile_pool(name="ps512", bufs=2, space="PSUM") as ps512,
            tc.tile_pool(name="psT", bufs=2, space="PSUM") as psT_pool,
            tc.tile_pool(name="b_kin", bufs=2) as b_kin,
            tc.tile_pool(name="b_ksb", bufs=2) as b_ksb,
            tc.tile_pool(name="b_vt", bufs=2) as b_vt,
            tc.tile_pool(name="b_et", bufs=2) as b_et,
            tc.tile_pool(name="b_ob", bufs=2) as b_ob,
            tc.tile_pool(name="b_rs", bufs=4) as b_rs,
            tc.tile_pool(name="b_psc", bufs=2, space="PSUM") as b_psc,
            tc.tile_pool(name="b_po", bufs=2, space="PSUM") as b_po,
        ):
            wq_sb = wpool.tile([128, 2, HALF], f16)
            wk_sb = wpool.tile([128, 2, C], f16)
            wv_sb = wpool.tile([128, 2, HALF], f16)
            nc.gpsimd.dma_start(out=wq_sb, in_=wqT.rearrange("(cb c) i -> c cb i", c=128))
            nc.gpsimd.dma_start(out=wk_sb, in_=wkT.rearrange("(cb c) j -> c cb j", c=128))
            nc.gpsimd.dma_start(out=wv_sb, in_=wvT.rearrange("(cb c) i -> c cb i", c=128))
            sb_bq = wpool.tile([128, 1], f32)
            sb_bk = wpool.tile([128, 2, 1], f32)
            sb_bv = wpool.tile([128, 1], f32)
            nc.gpsimd.dma_start(out=sb_bq, in_=bq_h)
            nc.gpsimd.dma_start(out=sb_bk, in_=bk_f.rearrange("(jb j) one -> j jb one", j=128))
            nc.gpsimd.dma_start(out=sb_bv, in_=bv_h)
            ident = wpool.tile([128, 128], f16)
            make_identity(nc, ident)
            # SBUF-resident transposed q for the whole kernel: qT[j, jb, i, h]
            qT = wpool.tile([128, 2, HALF, C], f16)

            def body(_it=None):
                # ---------------- Phase A: q conv -> resident qT; v conv ------
                for t in range(HJ // A_CHUNK):
                    sl = slice(t * A_CHUNK, (t + 1) * A_CHUNK)
                    qc = a_in.tile([128, 2, A_CHUNK], f16, tag="qc")
                    nc.sync.dma_start(out=qc, in_=qv_in[:, :, sl])
                    vc = a_in.tile([128, 2, A_CHUNK], f16, tag="vc")
                    nc.scalar.dma_start(out=vc, in_=vv_in[:, :, sl])
                    # q and v subchunks alternate so their PSUM drains
                    # land on different engines (q->DVE, v->ACT): the ps512
                    # ring turns over at 2x the single-engine drain rate and
                    # the PE stays continuously busy (p-state)
                    qss = []
                    vs = a_vs.tile([128, A_CHUNK], bf16, tag="vs")
                    for n in range(NSUB):
                        ps = ps512.tile([128, 512], f32, tag="aps")
                        for cb in range(2):
                            nc.tensor.matmul(ps, wq_sb[:, cb, :],
                                             qc[:, cb, n * 512:(n + 1) * 512],
                                             start=(cb == 0), stop=(cb == 1))
                        qs = a_qs.tile([128, 512], f16, tag="qs")
                        nc.vector.tensor_scalar(
                            out=qs, in0=ps, scalar1=sb_bq, scalar2=None,
                            op0=mybir.AluOpType.add)
                        qss.append(qs)
                        ps = ps512.tile([128, 512], f32, tag="aps")
                        for cb in range(2):
                            nc.tensor.matmul(ps, wv_sb[:, cb, :],
                                             vc[:, cb, n * 512:(n + 1) * 512],
                                             start=(cb == 0), stop=(cb == 1))
                        nc.scalar.activation(
                            out=vs[:, n * 512:(n + 1) * 512], in_=ps,
                            func=Ident, bias=sb_bv)
                    nc.sync.dma_start(out=v_flat[:, sl], in_=vs)
                    for n in range(NSUB):
                        pt = psT_pool.tile([128, 2, 2, 128], f16, tag="pt")
                        for h2 in range(2):
                            for jb in range(2):
                                nc.tensor.transpose(
                                    pt[:, h2, jb, :],
                                    qss[n][:, h2 * 256 + jb * 128:
                                           h2 * 256 + (jb + 1) * 128],
                                    ident)
                        h0 = (t * NSUB + n) * 2
                        # copyback split across the two PSUM-capable engines;
                        # each copy covers both jb blocks: free dims (jb, i)
                        nc.vector.tensor_copy(
                            out=qT[:, :, :, h0],
                            in_=pt[:, 0, :, :])
                        nc.scalar.copy(
                            out=qT[:, :, :, h0 + 1],
                            in_=pt[:, 1, :, :])

                # ---------------- Phase B: k conv + attention -----------------
                def kc_load(ic):
                    kc = b_kin.tile([128, 2, KCH], f16, tag="kc")
                    nc.gpsimd.dma_start(
                        out=kc, in_=kv_in[:, :, ic * KCH:(ic + 1) * KCH])
                    return kc
                def vt_load(ic):
                    i0 = ic * ICHUNK
                    vt = b_vt.tile([128, ICHUNK, 2, C + 8], bf16, tag="vt")
                    nc.sync.dma_start(
                        out=vt[:, :, :, 0:C],
                        in_=v_scr[i0:i0 + ICHUNK].rearrange(
                            "io (jb j) w -> j io jb w", j=128))
                    nc.gpsimd.memset(vt[:, :, :, C:C + 1], 2.0)
                    return vt

                kc_pend = {0: kc_load(0), 1: kc_load(1)}
                vt_pend = {0: vt_load(0)}
                for ic in range(HALF // ICHUNK):
                    i0 = ic * ICHUNK
                    kc = kc_pend.pop(ic)
                    if ic + 2 < HALF // ICHUNK:
                        kc_pend[ic + 2] = kc_load(ic + 2)
                    vt = vt_pend.pop(ic)
                    if ic + 1 < HALF // ICHUNK:
                        vt_pend[ic + 1] = vt_load(ic + 1)
                    ksb = b_ksb.tile([128, 2, ICHUNK, C], f16, tag="ksb")
                    for jb in range(2):
                        for n in range(KCH // 512):
                            ps = ps512.tile([128, 512], f32, tag="aps")
                            for cb in range(2):
                                nc.tensor.matmul(
                                    ps, wk_sb[:, cb, jb * 128:(jb + 1) * 128],
                                    kc[:, cb, n * 512:(n + 1) * 512],
                                    start=(cb == 0), stop=(cb == 1))
                            # k drains alternate DVE/ACT (Identity is in
                            # every act table set - no table reload)
                            if jb == 0:
                                nc.vector.tensor_scalar(
                                    out=ksb[:, jb, n * 2:(n + 1) * 2, :],
                                    in0=ps, scalar1=sb_bk[:, jb, :],
                                    scalar2=None, op0=mybir.AluOpType.add)
                            else:
                                nc.scalar.activation(
                                    out=ksb[:, jb, n * 2:(n + 1) * 2, :],
                                    in_=ps, func=Ident, bias=sb_bk[:, jb, :])
                    ob = b_ob.tile([128, ICHUNK, 2, C], f16, tag="ob")
                    stage = []
                    for io in range(ICHUNK):
                        i_loc = i0 + io
                        psc = b_psc.tile([128, 2, C], f32, tag="psc")
                        for wb in range(2):
                            for jb in range(2):
                                nc.tensor.matmul(
                                    psc[:, wb, :],
                                    ksb[:, jb, io, wb * 128:(wb + 1) * 128],
                                    qT[:, jb, i_loc, :],
                                    start=(jb == 0), stop=(jb == 1))
                        et = b_et.tile([128, 2, C], bf16, tag="et")
                        nc.scalar.activation(
                            out=et, in_=psc,
                            func=mybir.ActivationFunctionType.Exp)
                        stage.append((io, et))
                    for io, et in stage:
                        for hb in range(2):
                            po = b_po.tile([128, C + 1], f32, tag="po")
                            for wb in range(2):
                                nc.tensor.matmul(
                                    po, et[:, wb, hb * 128:(hb + 1) * 128],
                                    vt[:, io, wb, 0:C + 1],
                                    start=(wb == 0), stop=(wb == 1))
                            rs = b_rs.tile([128, 1], f32, tag="rs")
                            nc.vector.reciprocal(out=rs, in_=po[:, C:C + 1])
                            # sigmoid(y) = 0.5*tanh(y/2)+0.5, y/2 = po*rs
                            # (rs = 0.5/rowsum via vt's 2.0-column; bv is
                            # already in v from the phase-A drain). One
                            # scaled tanh per (io,hb) replaces the DVE
                            # normalize pass + batched tanh.
                            nc.scalar.activation(
                                out=ob[:, io, hb, :], in_=po[:, 0:C],
                                func=mybir.ActivationFunctionType.Tanh,
                                scale=rs)
                    nc.gpsimd.tensor_scalar(
                        out=ob, in0=ob,
                        scalar1=0.5, scalar2=0.5,
                        op0=mybir.AluOpType.mult,
                        op1=mybir.AluOpType.add)
                    nc.gpsimd.dma_start(
                        out=out_b[i0:i0 + ICHUNK].rearrange(
                            "io (hb h) w -> h io hb w", h=128),
                        in_=ob)

            if repeat == 1:
                body()
            else:
                with tc.For_i(0, repeat, 1) as it:
                    body(it)

    nc.compile()
    _CACHE[key] = nc
    return nc


def make_in_maps(inputs):
    query = np.asarray(inputs["query"], dtype=np.float32)
    key_in = np.asarray(inputs["key_in"], dtype=np.float32)
    value = np.asarray(inputs["value"], dtype=np.float32)
    Wq = np.asarray(inputs["Wq"], dtype=np.float32)
    Wk = np.asarray(inputs["Wk"], dtype=np.float32)
    Wv = np.asarray(inputs["Wv"], dtype=np.float32)
    bq = np.asarray(inputs["bq"], dtype=np.float32)
    bk = np.asarray(inputs["bk"], dtype=np.float32)
    bv = np.asarray(inputs["bv"], dtype=np.float32)
    in_maps = []
    for core in range(N_CORES):
        b, g = core // 2, core % 2
        sl = slice(g * HALF, (g + 1) * HALF)
        in_maps.append({
            "query_b": np.ascontiguousarray(query[b], dtype=np.float16),
            "key_h": np.ascontiguousarray(key_in[b][:, sl, :], dtype=np.float16),
            "value_b": np.ascontiguousarray(value[b], dtype=np.float16),
            "wqT": np.ascontiguousarray(Wq[sl, :].T, dtype=np.float16),
            "wkT": np.ascontiguousarray(Wk.T, dtype=np.float16),
            "wvT": np.ascontiguousarray(Wv[sl, :].T, dtype=np.float16),
            "bq_h": np.ascontiguousarray(bq[sl].reshape(HALF, 1)),
            "bk_f": np.ascontiguousarray(bk.reshape(C, 1)),
            "bv_h": np.ascontiguousarray(bv[sl].reshape(HALF, 1)),
        })
    return in_maps


def kernel(query, key_in, value, Wq, bq, Wk, bk, Wv, bv):
    nc = build_nc()
    in_maps = make_in_maps(dict(query=query, key_in=key_in, value=value,
                                Wq=Wq, bq=bq, Wk=Wk, bk=bk, Wv=Wv, bv=bv))
    res = run_bass_kernel_spmd(nc, in_maps, core_ids=list(range(N_CORES)))
    out = np.empty((4, C, C, C), dtype=np.float32)
    for core in range(N_CORES):
        b, g = core // 2, core % 2
        out[b, g * HALF:(g + 1) * HALF] = res.results[core]["out_b"].astype(np.float32)
    return out
